# revision 42
# baseline (speedup 1.0000x reference)
"""Multi-head cross-attention (B=2, S=2048, D=1024, H=16) on 8 trn2 cores.

Sharding: core c -> (batch b = c//4, head-group g = c%4, 4 heads per group).
Tensor-parallel heads: wq/wk/wv column-sliced, wo row-sliced; partial outputs
summed on host.  Key-mask compaction on host: only unmasked keys are shipped
(padded to K_PAD), since masked keys contribute exactly zero after softmax.

v2 vs baseline:
- softmax sums folded into the AV matmul via a ones column appended to V
  (lhsT [128, 65] -> av psum [65, 512], row 64 = sums); the 144 separate
  ones-matmul streams are gone (-31us of PE).
- attention units ordered pair-major (qc, pair, kt) so only 2+1 av psum
  banks are needed, freeing a bank for the q-proj/out-proj "work" slot.
- normalization fused into psum evacuation: outT = av_psum * bc in one DVE
  tensor_tensor (probed: mixed partition bases legal when in0 is PSUM).
- fp16 output DMA (half the out traffic); host accumulates partials in f32.

v3 (this file) vs v2 — measured ~154us -> ~150us:
- 1/sums moved OFF the saturated ACT engine (its exp stream is the phase
  co-bottleneck at ~1us/unit): fast-inverse bit trick + one Newton step on
  the DVE (3 ops on [33, 512]); the Newton sign flip is absorbed by a
  negated broadcast selector e2n.  The serial tail keeps Ln/Exp on ACT
  (idle there).  Max 1/s error ~6e-3 -> rel err 1.9e-3 (gate 2e-2).
- EARLY-START schedule: only k(pair0, c0) + q0(pair0) run as prologue;
  the other 15 projection groups are interleaved one-per-unit into the
  attention stream (ordered by DMA arrival), so the first exp fires at
  ~18us instead of ~48us.  qc0's units follow memT chunk arrival
  (kt-blocks of 3, both pairs per block).  AV flushes are DEFERRED while
  projections own the "av" psum rotation (ppool holds the p_t backlog),
  then drain 2/unit grouped by pair -- AV accumulation order over kt is
  irrelevant, only the per-pair first/last flags matter.
- input DMA reordered for the early-start critical path (mb/bq/wk/xc0/wq
  lead, then memT, wv, late xc chunks); wq/wo split across both queues.
- PE p-state warmup: the clock sits at 1.2GHz until a few dozen matmuls
  retire, so 64 dummy matmuls burn the initial DMA window and the real
  prologue starts at full rate.
- epilogue generators advance 2 steps/unit so av banks recycle faster at
  pair transitions.

Known-remaining (measured, resisted 3 scheduling attempts each): ~8us of
qc-boundary ACT stalls (out-proj PE surplus), ~10us endgame at half clock
(p-state drops in the serial tail; dummy-matmul keep-alive backfired by
serializing into the dependency chain), ~7us NEFF semaphore-reset
ceremony (compiler-inserted, one inst per ~256 sems).

Algebraic folds (exact): bk shifts every logit of a query by a constant ->
softmax-invariant -> dropped.  bv adds sum(weights)*bv = bv to every attention
output -> (bv @ wo + bo) added on host.
"""

import math
import sys
import types
from contextlib import ExitStack

import numpy as np

# --- shim antenv.axon_hooks so trace=True works under axon -----------------
if "antenv.axon_hooks" not in sys.modules:
    _mod = types.ModuleType("antenv.axon_hooks")
    _hook_box = [None]
    _mod.set_axon_ntff_profile_hook = lambda h: _hook_box.__setitem__(0, h)
    _mod.get_axon_ntff_profile_hook = lambda: _hook_box[0]
    sys.modules["antenv.axon_hooks"] = _mod
    try:
        import antenv

        antenv.axon_hooks = _mod
        from trn_agent_boot.trn_boot import _ntff_profile_via_ctypes

        _mod.set_axon_ntff_profile_hook(
            _ntff_profile_via_ctypes("/opt/axon/libaxon_pjrt.so")
        )
    except Exception:
        pass

import concourse.bass as bass
import concourse.mybir as mybir
import concourse.tile as tile
from concourse.bass_utils import run_bass_kernel_spmd
from concourse.vector_clock import ScopedClock

# --- patch Tile tail drain: this walrus build rejects CTRL insts with >1-2
# sem waits ("Too many sync wait commands").  Split the tail drain's waits
# into one drain per outstanding proc tick.
def _drain_and_barrier_split(self, tick_clock, wait_clock):
    nc = self.nc
    g = ScopedClock({None: tick_clock.global_clock})
    for scope, vc in g.items():
        for proc in range(len(vc)):
            t = vc[proc]
            if t > 0:
                sc = ScopedClock()
                sc.require_at_least(scope, proc, t)
                d = nc.sync.drain()
                wait_clock.add_sem_waits(d.ins, sc)
    nc.all_engine_barrier()
    assert self.sems is not None
    popped = nc._tile_sem_poison_stack.pop()
    assert popped is self._sem_poison
    nc.clear_and_free_semaphores(list(self.sems.allocated().values()))
    nc.all_engine_barrier()


tile.TileContext._drain_and_barrier = _drain_and_barrier_split

# This walrus build tolerates only 1 sync wait per instruction.  Hoist excess
# waits onto preceding EVENT_SEMAPHORE nops (the native wait_ge carrier).
_MAX_WAITS = 1
_orig_lower = tile.TileContext._lower_ordered_insts


def _is_self_wait(inst, w):
    # A ge-wait on the instruction's own engine sem is transitively implied
    # by in-order execution (Tile's vector clock is not transitively minimal).
    if w.wait_mode != "sem-ge-imm" or not w.ant_name:
        return False
    eng = str(inst.engine).split(".")[-1]
    return w.ant_name.startswith(eng + "_")


def _lower_split_waits(self, ordered):
    nc = self.nc
    for bb_name, insts in ordered.items():
        out = []
        for inst in insts:
            si = inst.sync_info
            if si is not None and si.on_wait:
                waits = [w for w in si.on_wait if not _is_self_wait(inst, w)]
                if len(waits) != len(si.on_wait) or len(waits) > _MAX_WAITS:
                    excess, keep = waits[:-_MAX_WAITS], waits[-_MAX_WAITS:]
                    for w in excess:
                        d = mybir.InstEventSemaphore(
                            name=nc.get_next_instruction_name(), ins=[], outs=[]
                        )
                        d.engine = inst.engine
                        d.sync_info = mybir.SyncInfo(on_wait=[w], on_update=[])
                        out.append(d)
                    inst.sync_info = mybir.SyncInfo(
                        on_wait=keep, on_update=list(si.on_update)
                    )
            out.append(inst)
        insts[:] = out
    return _orig_lower(self, ordered)


tile.TileContext._lower_ordered_insts = _lower_split_waits

F32 = mybir.dt.float32
F16 = mybir.dt.float16
B, S, D, H = 2, 2048, 1024, 16
DH = 64
G = 4  # head-groups == cores per batch
CD = D // G  # 256 head dims per core (4 heads)
N_CORES = 8
NEG = -1.0e30
K_PAD_LADDER = (1152, 1536, 2304)  # multiples of MEMC=384
Exp = mybir.ActivationFunctionType.Exp
Ln = mybir.ActivationFunctionType.Ln


def _chunks(total, sz):
    out = []
    off = 0
    while off < total:
        c = min(sz, total - off)
        out.append((off, c))
        off += c
    return out


MEMC = 384  # memT column-chunk width (k_pad must be divisible)


def _build(k_pad: int) -> bass.Bass:
    kt_tiles = k_pad // 128
    n_mc = k_pad // MEMC

    nc = bass.Bass()
    # All inputs are host-pre-shuffled so every DMA is a contiguous slab
    # (strided rearranges cost ~2.4us of engine issue time each).
    xT = nc.dram_tensor("xT", [2, 4, 128, 4, 512], F16, kind="ExternalInput")
    memT = nc.dram_tensor("memT", [2, n_mc, 128, 4, MEMC], F16, kind="ExternalInput")
    wq_d = nc.dram_tensor("wq", [128, 8, CD], F16, kind="ExternalInput")
    wk_d = nc.dram_tensor("wk", [128, 8, CD], F16, kind="ExternalInput")
    wv_d = nc.dram_tensor("wv", [128, 8, CD], F16, kind="ExternalInput")
    bq_d = nc.dram_tensor("bq", [128, 2], F32, kind="ExternalInput")
    wo_d = nc.dram_tensor("wo", [128, 2, D], F16, kind="ExternalInput")
    mb_d = nc.dram_tensor("maskb", [128, kt_tiles], F32, kind="ExternalInput")
    out_d = nc.dram_tensor("out", [S, D], F16, kind="ExternalOutput")
    # pair0's partial out-projection of the LAST q-chunk (host adds it onto
    # out rows QOFF[-1]:) -- lets that half run before pair1 finishes.
    out2_d = nc.dram_tensor("out2", [512, D], F16, kind="ExternalOutput")

    with tile.TileContext(nc) as tc, ExitStack() as ctx:
        consts = ctx.enter_context(tc.tile_pool(name="consts", bufs=1))
        bigin = ctx.enter_context(tc.tile_pool(name="bigin", bufs=1))
        wpool = ctx.enter_context(tc.tile_pool(name="wp", bufs=1))
        qkv = ctx.enter_context(tc.tile_pool(name="qkv", bufs=1))
        ppool = ctx.enter_context(tc.tile_pool(name="pp", bufs=22))
        npool = ctx.enter_context(tc.tile_pool(name="np", bufs=2))
        opool = ctx.enter_context(tc.tile_pool(name="op", bufs=4))
        # PSUM: lt 2x[128,1024] (4 banks) + av 3x[65->128,512] (3) + work 1
        psum = ctx.enter_context(tc.tile_pool(name="ps", bufs=1, space="PSUM"))

        # E2 selector for the 1/s broadcast: bc rows 0:64 <- rcp row 0,
        # rows 64:128 <- rcp row 32.  e2 (+1) for the tail's Exp(-Ln) path,
        # e2n (-1) for the DVE Newton path (which yields -1/s).
        e2 = consts.tile([33, 128], F16, tag="E2")
        nc.vector.memset(e2, 0.0)
        nc.vector.memset(e2[0:1, 0:64], 1.0)
        nc.vector.memset(e2[32:33, 64:128], 1.0)
        e2n = consts.tile([33, 128], F16, tag="E2N")
        nc.vector.memset(e2n, 0.0)
        nc.vector.memset(e2n[0:1, 0:64], -1.0)
        nc.vector.memset(e2n[32:33, 64:128], -1.0)
        # magic constant for the fast-inverse seed: bits 0x7EF477D5
        mgc = consts.tile([33, 512], F32, tag="MGC")
        nc.vector.memset(mgc, 1.6247691722546535e+38)

        bq_sb = consts.tile([128, 2], F32, tag="bq")
        mb_sb = consts.tile([128, kt_tiles], F32, tag="mb")
        # touch Exp+Ln once so the ACT table loads happen during the DMA wait,
        # not in front of the first real exp
        warm = consts.tile([1, 2], F32, tag="warm")
        nc.vector.memset(warm, 1.0)
        nc.scalar.activation(warm[0:1, 0:1], warm[0:1, 1:2], Exp)
        nc.scalar.activation(warm[0:1, 0:1], warm[0:1, 1:2], Ln)
        # spin the PE during the input-DMA wait: the clock ramps with
        # activity (~1.2GHz until a few dozen matmuls retire), so burn the
        # idle window on dummies instead of running the real prologue at
        # half rate.
        wm_ps = psum.tile([128, 512], F32, tag="work", bufs=1, name="wm_ps")
        for _ in range(64):
            nc.tensor.matmul(wm_ps[:, 0:128], e2, e2, start=True, stop=True)

        wk_sb = wpool.tile([128, 8, CD], F16, tag="wk")
        wv_sb = wpool.tile([128, 8, CD], F16, tag="wv")
        wq_sb = wpool.tile([128, 8, CD], F16, tag="wq")
        wo_sb = wpool.tile([128, 2, D], F16, tag="wo")
        memc = [
            [
                bigin.tile([128, 4, MEMC], F16, tag=f"memc{h}_{c}", name=f"memc{h}_{c}")
                for c in range(n_mc)
            ]
            for h in range(2)
        ]
        xc = [
            [
                bigin.tile([128, 4, 512], F16, tag=f"xc{h}_{q}", name=f"xc{h}_{q}")
                for q in range(4)
            ]
            for h in range(2)
        ]

        # --- input DMA (all contiguous slabs), ordered for the EARLY-START
        # critical path: the attention unit stream begins as soon as
        # wk + wq + xc[*][0] + memc[*][0] + mb/bq have landed (~15us), so
        # those lead both queues; memT/wv/late xc chunks follow.
        nc.scalar.dma_start(out=mb_sb, in_=mb_d[:, :])
        nc.scalar.dma_start(out=bq_sb, in_=bq_d[:, :])
        nc.sync.dma_start(out=wk_sb[:, 0:4, :], in_=wk_d[:, 0:4, :])
        nc.scalar.dma_start(out=wk_sb[:, 4:8, :], in_=wk_d[:, 4:8, :])
        nc.sync.dma_start(out=xc[0][0], in_=xT[0, 0])
        nc.scalar.dma_start(out=xc[1][0], in_=xT[1, 0])
        nc.sync.dma_start(out=wq_sb[:, 0:4, :], in_=wq_d[:, 0:4, :])
        nc.scalar.dma_start(out=wq_sb[:, 4:8, :], in_=wq_d[:, 4:8, :])
        for c in range(n_mc):
            nc.sync.dma_start(out=memc[0][c], in_=memT[0, c])
            nc.scalar.dma_start(out=memc[1][c], in_=memT[1, c])
        nc.scalar.dma_start(out=wv_sb, in_=wv_d[:, :, :])
        nc.sync.dma_start(out=xc[0][1], in_=xT[0, 1])
        nc.scalar.dma_start(out=xc[1][1], in_=xT[1, 1])
        nc.sync.dma_start(out=wo_sb[:, 0, :], in_=wo_d[:, 0, :])
        nc.scalar.dma_start(out=wo_sb[:, 1, :], in_=wo_d[:, 1, :])
        for q in range(2, 4):
            nc.sync.dma_start(out=xc[0][q], in_=xT[0, q])
            nc.scalar.dma_start(out=xc[1][q], in_=xT[1, q])

        # --- K/V/Q0 projections (psum via the "av" 3-slot rotation) --------
        # query chunks: last 512 split in two so the drain tail is half-sized
        QW = (512, 512, 512, 512)
        QOFF = (0, 512, 1024, 1536)
        NQ = len(QW)
        QT = [
            [
                qkv.tile([128, QW[q]], F16, tag=f"QT{p}_{q}", name=f"QT{p}_{q}")
                for q in range(NQ)
            ]
            for p in range(2)
        ]
        KT = [qkv.tile([128, k_pad], F16, tag=f"KT{p}", name=f"KT{p}") for p in range(2)]
        # V with a ones column per head: [128 keys, head, 64 dims + 1]
        V = [
            qkv.tile([128, 4, 65], F16, tag=f"V{kt}", name=f"V{kt}")
            for kt in range(kt_tiles)
        ]
        for kt in range(kt_tiles):
            nc.vector.memset(V[kt][:, :, 64:65], 1.0)

        def k_chunk(pair, c, tag, bufs):
            cs = slice(128 * pair, 128 * pair + 128)
            k_ps = psum.tile([128, MEMC], F32, tag=tag, bufs=bufs, name="k_ps")
            for dt in range(8):
                nc.tensor.matmul(
                    k_ps,
                    wk_sb[:, dt, cs],
                    memc[dt // 4][c][:, dt % 4, :],
                    start=(dt == 0),
                    stop=(dt == 7),
                )
            nc.vector.tensor_copy(KT[pair][:, c * MEMC : (c + 1) * MEMC], k_ps)

        def v_chunk(kt, tag, bufs):
            c, coff = divmod(kt * 128, MEMC)
            v_ps = psum.tile([128, 4, 64], F32, tag=tag, bufs=bufs, name="v_ps")
            for dt in range(8):
                nc.tensor.matmul(
                    v_ps,
                    memc[dt // 4][c][:, dt % 4, coff : coff + 128],
                    wv_sb[:, dt, :],
                    start=(dt == 0),
                    stop=(dt == 7),
                )
            nc.vector.tensor_copy(V[kt][:, :, 0:64], v_ps)

        def q0_chunk(pair, tag, bufs):
            cs = slice(128 * pair, 128 * pair + 128)
            q_ps = psum.tile([128, 512], F32, tag=tag, bufs=bufs, name="q_ps")
            for dt in range(8):
                nc.tensor.matmul(
                    q_ps,
                    wq_sb[:, dt, cs],
                    xc[dt // 4][0][:, dt % 4, :],
                    start=(dt == 0),
                    stop=(dt == 7),
                )
            nc.vector.tensor_scalar_add(QT[pair][0], q_ps, bq_sb[:, pair : pair + 1])

        # minimal prologue: just what unit (qc0, pair0, kt0) needs.  The
        # remaining projection groups are interleaved one-per-unit into the
        # early attention stream (their psum "av" rotation is free because AV
        # flushes are deferred until the projections complete).  Group order
        # follows DMA arrival so a stalled group never blocks a ready QK.
        q0_chunk(0, "av", 3)  # q0 inputs land ~2us before memc: keep PE fed
        k_chunk(0, 0, "av", 3)
        proj = [
            lambda: k_chunk(1, 0, "av", 3),
            lambda: q0_chunk(1, "av", 3),
        ]
        for c in range(1, n_mc):
            proj.append(lambda c=c: k_chunk(0, c, "av", 3))
            proj.append(lambda c=c: k_chunk(1, c, "av", 3))
        proj.extend(
            lambda kt=kt: v_chunk(kt, "av", 3) for kt in range(kt_tiles)
        )

        def q_stream_gen(qqc):
            # one q-chunk through the "work" psum slot; fine-grained 4-dt steps
            w, off = QW[qqc], QOFF[qqc]
            blk, boff = divmod(off, 512)
            for pair in range(2):
                cs = slice(128 * pair, 128 * pair + 128)
                q_ps = psum.tile([128, 512], F32, tag="work", bufs=1, name="q_ps")
                for dt in range(8):
                    nc.tensor.matmul(
                        q_ps[:, 0:w],
                        wq_sb[:, dt, cs],
                        xc[dt // 4][blk][:, dt % 4, boff : boff + w],
                        start=(dt == 0),
                        stop=(dt == 7),
                    )
                    if dt == 3:
                        yield
                nc.vector.tensor_scalar_add(
                    QT[pair][qqc], q_ps[:, 0:w], bq_sb[:, pair : pair + 1]
                )
                yield

        # --- attention: flat software-pipelined stream, pair-major ---------
        outT = [
            [
                qkv.tile([128, QW[q]], F16, tag=f"oT{p}_{q}", name=f"oT{p}_{q}")
                for q in range(NQ)
            ]
            for p in range(2)
        ]

        st_av = {}  # (qc, pair) -> [avA, avB] psum tiles [65, w]
        epi_count = [0]
        completed = []  # (qc, pair) whose final AV flush has been emitted
        nflush = {}  # (qc, pair) -> AV flushes emitted so far
        group_order = []  # (qc, pair) in first-push order
        group_seen = set()

        def flush(pend):
            # AV accumulation is order-independent over kt: start/stop flags
            # follow the per-pair flush COUNT, not the kt index.
            qc, pair, kt, p_t = pend
            w = QW[qc]
            n = nflush.get((qc, pair), 0)
            nflush[(qc, pair)] = n + 1
            av = st_av.setdefault((qc, pair), [None, None])
            for a in range(2):
                if av[a] is None:
                    av[a] = psum.tile(
                        [65, 512], F32, tag="av", bufs=3, name=f"av{pair}_{a}"
                    )[:, 0:w]
                nc.tensor.matmul(
                    av[a],
                    V[kt][:, 2 * pair + a, :],
                    p_t[:, a, :],
                    start=(n == 0),
                    stop=(n == kt_tiles - 1),
                )
            if n == kt_tiles - 1:
                completed.append((qc, pair))

        def drain_one(pending):
            # Flush only the oldest still-accumulating pair: at most two pairs
            # may hold av psum banks at once (3-slot rotation).
            g = next(
                (k for k in group_order if nflush.get(k, 0) < kt_tiles), None
            )
            idx = next(
                (i for i, p in enumerate(pending) if (p[0], p[1]) == g), None
            )
            if idx is None:
                return False
            flush(pending.pop(idx))
            return True

        def pair_epilogue(qc, pair):
            """1/s via Ln+Exp, E2-broadcast, fused normalize; out-proj on pair 1."""
            w, off = QW[qc], QOFF[qc]
            tail = qc == NQ - 1 and pair == 1  # nothing left to overlap with
            avA, avB = st_av[(qc, pair)]
            g = npool.tile([33, 512], F32, tag="g", bufs=4)
            if epi_count[0] < 4:
                nc.vector.memset(g, 1.0)  # keep Ln input positive in slack rows
            epi_count[0] += 1

            def outproj(sl, wtag, wbufs, ev_alt, cts=(0, 1)):
                o_sb = opool.tile([128, 1024], F16, tag="osb", name="o_sb")
                for nch in range(2):
                    o_ps = psum.tile(
                        [128, 512], F32,
                        tag=(wtag if nch % 2 else "work"),
                        bufs=(wbufs if nch % 2 else 1),
                        name="o_ps",
                    )
                    for j, ct in enumerate(cts):
                        nc.tensor.matmul(
                            o_ps,
                            outT[ct][qc][:, sl * 128 : (sl + 1) * 128],
                            wo_sb[:, ct, nch * 512 : (nch + 1) * 512],
                            start=(j == 0),
                            stop=(j == len(cts) - 1),
                        )
                    if ev_alt and nch % 2:
                        nc.scalar.copy(o_sb[:, nch * 512 : (nch + 1) * 512], o_ps)
                    else:
                        nc.vector.tensor_copy(
                            o_sb[:, nch * 512 : (nch + 1) * 512], o_ps
                        )
                return o_sb

            if tail:
                # final 256-q chunk: short serial chain, scalar engine takes
                # the gathers + odd evacs since it is idle here
                nc.scalar.copy(g[0:1, 0:w], avA[64:65, :])
                nc.scalar.copy(g[32:33, 0:w], avB[64:65, :])
                lns = npool.tile([33, w], F32, tag="lns", bufs=4)
                nc.scalar.activation(lns, g[:, 0:w], Ln)
                rcp16 = npool.tile([33, w], F16, tag="rcp16", bufs=4)
                nc.scalar.activation(rcp16, lns, Exp, scale=-1.0)
                bc_ps = psum.tile([128, w], F32, tag="work", bufs=1, name="bc_ps")
                nc.tensor.matmul(bc_ps, e2, rcp16, start=True, stop=True)
                bc_sb = npool.tile([128, w], F32, tag="bc_sb", bufs=4)
                nc.vector.tensor_copy(bc_sb, bc_ps)
                nc.vector.tensor_tensor(
                    outT[pair][qc][0:64, :], avA[0:64, :], bc_sb[0:64, :],
                    op=mybir.AluOpType.mult,
                )
                nc.vector.tensor_tensor(
                    outT[pair][qc][64:128, :], avB[0:64, :], bc_sb[64:128, :],
                    op=mybir.AluOpType.mult,
                )
                for sl in range(w // 128):
                    o_sb = outproj(sl, "lt", 2, ev_alt=True, cts=(1,))
                    st = off // 128 + sl
                    eng = nc.scalar if sl % 2 else nc.sync
                    eng.dma_start(out=out_d[st * 128 : (st + 1) * 128, :], in_=o_sb)
                del st_av[(qc, pair)]
                return

            nc.vector.tensor_copy(g[0:1, 0:w], avA[64:65, :])
            nc.vector.tensor_copy(g[32:33, 0:w], avB[64:65, :])
            yield
            # 1/s on the DVE via fast-inverse seed + 2 Newton steps, keeping
            # the scalar engine free for the exp stream.  x0 = bitcast(MAGIC -
            # bitcast(s)); xk alternates sign: the final value is -1/s,
            # absorbed by the negated e2n selector.
            U32 = mybir.dt.uint32
            x0 = npool.tile([33, w], F32, tag="lns", bufs=4)
            nc.vector.tensor_tensor(
                x0.bitcast(U32), mgc[:, 0:w].bitcast(U32), g[:, 0:w].bitcast(U32),
                op=mybir.AluOpType.subtract,
            )
            t1 = npool.tile([33, w], F32, tag="nt", bufs=2)
            nc.vector.tensor_tensor(t1, g[:, 0:w], x0, op=mybir.AluOpType.mult)
            yield
            rcp16 = npool.tile([33, w], F16, tag="rcp16", bufs=4)
            nc.vector.scalar_tensor_tensor(
                rcp16, t1, 2.0, x0,
                op0=mybir.AluOpType.subtract, op1=mybir.AluOpType.mult,
            )
            yield
            bc_ps = psum.tile([128, w], F32, tag="work", bufs=1, name="bc_ps")
            nc.tensor.matmul(bc_ps, e2n, rcp16, start=True, stop=True)
            bc_sb = npool.tile([128, w], F32, tag="bc_sb", bufs=4)
            nc.vector.tensor_copy(bc_sb, bc_ps)
            yield
            nc.vector.tensor_tensor(
                outT[pair][qc][0:64, :], avA[0:64, :], bc_sb[0:64, :],
                op=mybir.AluOpType.mult,
            )
            nc.vector.tensor_tensor(
                outT[pair][qc][64:128, :], avB[0:64, :], bc_sb[64:128, :],
                op=mybir.AluOpType.mult,
            )
            del st_av[(qc, pair)]
            yield
            if pair == 0:
                if qc == NQ - 1:
                    # last chunk: pair0's half of the out-projection runs NOW
                    # (overlapping pair1's remaining units), DMA'd to a
                    # separate partial buffer the host adds in.  The serial
                    # tail then only carries pair1's half.
                    for sl in range(w // 128):
                        o_sb = outproj(sl, "work", 1, ev_alt=False, cts=(0,))
                        yield
                        eng = nc.scalar if sl % 2 else nc.sync
                        eng.dma_start(
                            out=out2_d[sl * 128 : (sl + 1) * 128, :], in_=o_sb
                        )
                return
            for sl in range(w // 128):
                o_sb = outproj(sl, "work", 1, ev_alt=False)
                yield
                st = off // 128 + sl
                nc.sync.dma_start(out=out_d[st * 128 : (st + 1) * 128, :], in_=o_sb)

        # qc0's units run during the projection interleave, so they follow
        # memT chunk arrival (kt blocks of 3, both pairs per block); later
        # chunks are pair-major as before.
        units = [
            (0, pair, 3 * blk + k)
            for blk in range(n_mc)
            for pair in range(2)
            for k in range(3)
        ]
        units += [
            (qc, pair, kt)
            for qc in range(1, NQ)
            for pair in range(2)
            for kt in range(kt_tiles)
        ]

        def _step2(gens):
            # two steps per unit: halves epilogue wall-latency so av psum
            # banks recycle sooner at pair transitions
            out = []
            for g_ in gens:
                if next(g_, StopIteration) is StopIteration:
                    continue
                if next(g_, StopIteration) is StopIteration:
                    continue
                out.append(g_)
            return out

        prio = []
        # q-chunk streams gated into the PE-slack zone (chunk qi's QT is
        # needed at unit qi*18); chunk 1 waits for its xc DMA (~unit 3).
        qgens = [
            (q_stream_gen(1), 8),
            (q_stream_gen(2), 22),
            (q_stream_gen(3), 40),
        ]
        pending = []
        for u, (qc, pair, kt) in enumerate(units):
            # QK first: the tensor queue head must be the op that feeds the
            # exp stream, never something that waits on it.
            w = QW[qc]
            ks = slice(kt * 128, (kt + 1) * 128)
            # psum accumulation groups must stay bank-aligned: keep the two
            # QK halves in separate 2KB banks even for 256-wide chunks
            lt_full = psum.tile([128, 2, 512], F32, tag="lt", bufs=2, name="lt")
            lt = lt_full[:, :, 0:w]
            nc.tensor.matmul(
                lt[:, 0, :], KT[pair][0:64, ks], QT[pair][qc][0:64, :],
                start=True, stop=True,
            )
            nc.tensor.matmul(
                lt[:, 1, :], KT[pair][64:128, ks], QT[pair][qc][64:128, :],
                start=True, stop=True,
            )
            p_full = ppool.tile([128, 2, 512], F16, tag="p")
            p_t = p_full[:, :, 0:w]
            nc.scalar.activation(
                p_t, lt, Exp, bias=mb_sb[:, kt : kt + 1], scale=0.125
            )
            if (qc, pair) not in group_seen:
                group_seen.add((qc, pair))
                group_order.append((qc, pair))
            pending.append((qc, pair, kt, p_t))
            if proj:
                # one deferred projection group per unit; AV flushes wait
                # (ppool is deep enough to hold the p_t backlog)
                proj.pop(0)()
            else:
                # drain the flush backlog at up to 2/unit, then steady-state
                # with a 4-unit lag (the lag keeps the in-order tensor queue
                # from stalling on an exp still in flight)
                drains = 0
                while len(pending) > 4 and drains < 2:
                    if not drain_one(pending):
                        break
                    drains += 1
            for key in completed:
                prio.append(pair_epilogue(*key))
            completed.clear()
            while qgens and qgens[0][1] <= u:
                if next(qgens[0][0], StopIteration) is StopIteration:
                    qgens.pop(0)
                break
            prio = _step2(prio)

        while pending:
            assert drain_one(pending)
            for key in completed:
                prio.append(pair_epilogue(*key))
            completed.clear()
            prio = _step2(prio)
        for gen, _ in qgens:
            for _ in gen:
                pass
        for gen in prio:
            for _ in gen:
                pass

    return nc


_PROG_CACHE: dict[int, bass.Bass] = {}


def _get_prog(k_pad: int) -> bass.Bass:
    if k_pad not in _PROG_CACHE:
        _PROG_CACHE[k_pad] = _build(k_pad)
    return _PROG_CACHE[k_pad]


def _shuf_w(w):
    # [1024, CD] -> [128, 8, CD]: sbuf partition p holds rows 128*t + p
    return np.ascontiguousarray(w.astype(np.float16).reshape(8, 128, -1).transpose(1, 0, 2))


def _shuf_chunks(tT, csz):
    # [D, N] -> [2, N/csz, 128, 4, csz] contiguous DMA slabs per (half, chunk)
    d, n = tT.shape
    a = tT.reshape(2, 4, 128, n // csz, csz)  # [half, t, p, chunk, csz]
    return np.ascontiguousarray(a.transpose(0, 3, 2, 1, 4))


def _prep_inputs(x, memory, mask, wq, bq, wk, wv, k_pad):
    """Build the 8 per-core input maps."""
    kt_tiles = k_pad // 128
    in_maps = []
    per_batch = []
    for b in range(B):
        idx = np.flatnonzero(~mask[b])
        n = len(idx)
        assert n <= k_pad
        mem_g = np.zeros((k_pad, D), np.float16)
        mem_g[:n] = memory[b][idx].astype(np.float16)
        memT_b = _shuf_chunks(np.ascontiguousarray(mem_g.T), MEMC)
        xT_b = _shuf_chunks(np.ascontiguousarray(x[b].astype(np.float16).T), 512)
        mbias = np.zeros(k_pad, np.float32)
        mbias[n:] = NEG
        mb_b = np.ascontiguousarray(mbias.reshape(kt_tiles, 128).T)
        per_batch.append((xT_b, memT_b, mb_b, idx))
    for c in range(N_CORES):
        b, g = divmod(c, G)
        xT_b, memT_b, mb_b, _ = per_batch[b]
        cs = slice(g * CD, (g + 1) * CD)
        in_maps.append(
            {
                "xT": xT_b,
                "memT": memT_b,
                "wq": _shuf_w(wq[:, cs]),
                "wk": _shuf_w(wk[:, cs]),
                "wv": _shuf_w(wv[:, cs]),
                "bq": np.ascontiguousarray(bq[cs].reshape(2, 128).T.astype(np.float32)),
                "wo": None,  # filled by caller (needs wo)
                "maskb": mb_b,
            }
        )
    return in_maps, per_batch


def kernel(x, memory, mask, wq, bq, wk, bk, wv, bv, wo, bo, _trace=False):
    x = np.asarray(x, np.float32)
    memory = np.asarray(memory, np.float32)
    mask = np.asarray(mask).astype(bool)
    wq = np.asarray(wq, np.float32)
    bq = np.asarray(bq, np.float32)
    wk = np.asarray(wk, np.float32)
    wv = np.asarray(wv, np.float32)
    bv = np.asarray(bv, np.float32)
    wo = np.asarray(wo, np.float32)
    bo = np.asarray(bo, np.float32)

    nmax = max(int((~mask[b]).sum()) for b in range(B))
    k_pad = next(k for k in K_PAD_LADDER if k >= nmax)
    prog = _get_prog(k_pad)

    in_maps, _ = _prep_inputs(x, memory, mask, wq, bq, wk, wv, k_pad)
    for c in range(N_CORES):
        g = c % G
        in_maps[c]["wo"] = np.ascontiguousarray(
            wo[g * CD : (g + 1) * CD, :].astype(np.float16).reshape(2, 128, D).transpose(1, 0, 2)
        )

    res = run_bass_kernel_spmd(prog, in_maps, list(range(N_CORES)), trace=_trace)
    outs = [res.results[c]["out"] for c in range(N_CORES)]
    outs2 = [res.results[c]["out2"] for c in range(N_CORES)]
    final = np.empty((B, S, D), np.float32)
    tail = bo + bv @ wo
    for b in range(B):
        final[b] = outs[G * b].astype(np.float32)
        for g in range(1, G):
            final[b] += outs[G * b + g].astype(np.float32)
        for g in range(G):
            final[b][S - 512 :] += outs2[G * b + g].astype(np.float32)
        final[b] += tail[None, :]
    if _trace:
        kernel.last_exec_time_ns = res.exec_time_ns
    return final



# revision 44
# speedup vs baseline: 1.0070x; 1.0070x over previous
"""Multi-head cross-attention (B=2, S=2048, D=1024, H=16) on 8 trn2 cores.

Sharding: core c -> (batch b = c//4, head-group g = c%4, 4 heads per group).
Tensor-parallel heads: wq/wk/wv column-sliced, wo row-sliced; partial outputs
summed on host.  Key-mask compaction on host: only unmasked keys are shipped
(padded to K_PAD), since masked keys contribute exactly zero after softmax.

v2 vs baseline:
- softmax sums folded into the AV matmul via a ones column appended to V
  (lhsT [128, 65] -> av psum [65, 512], row 64 = sums); the 144 separate
  ones-matmul streams are gone (-31us of PE).
- attention units ordered pair-major (qc, pair, kt) so only 2+1 av psum
  banks are needed, freeing a bank for the q-proj/out-proj "work" slot.
- normalization fused into psum evacuation: outT = av_psum * bc in one DVE
  tensor_tensor (probed: mixed partition bases legal when in0 is PSUM).
- fp16 output DMA (half the out traffic); host accumulates partials in f32.

v3 (this file) vs v2 — measured ~154us -> ~150us:
- 1/sums moved OFF the saturated ACT engine (its exp stream is the phase
  co-bottleneck at ~1us/unit): fast-inverse bit trick + one Newton step on
  the DVE (3 ops on [33, 512]); the Newton sign flip is absorbed by a
  negated broadcast selector e2n.  The serial tail keeps Ln/Exp on ACT
  (idle there).  Max 1/s error ~6e-3 -> rel err 1.9e-3 (gate 2e-2).
- EARLY-START schedule: only k(pair0, c0) + q0(pair0) run as prologue;
  the other 15 projection groups are interleaved one-per-unit into the
  attention stream (ordered by DMA arrival), so the first exp fires at
  ~18us instead of ~48us.  qc0's units follow memT chunk arrival
  (kt-blocks of 3, both pairs per block).  AV flushes are DEFERRED while
  projections own the "av" psum rotation (ppool holds the p_t backlog),
  then drain 2/unit grouped by pair -- AV accumulation order over kt is
  irrelevant, only the per-pair first/last flags matter.
- input DMA reordered for the early-start critical path (mb/bq/wk/xc0/wq
  lead, then memT, wv, late xc chunks); wq/wo split across both queues.
- PE p-state warmup: the clock sits at 1.2GHz until a few dozen matmuls
  retire, so 64 dummy matmuls burn the initial DMA window and the real
  prologue starts at full rate.
- epilogue generators advance 2 steps/unit so av banks recycle faster at
  pair transitions.

Known-remaining (measured, resisted 3 scheduling attempts each): ~8us of
qc-boundary ACT stalls (out-proj PE surplus), ~10us endgame at half clock
(p-state drops in the serial tail; dummy-matmul keep-alive backfired by
serializing into the dependency chain), ~7us NEFF semaphore-reset
ceremony (compiler-inserted, one inst per ~256 sems).

Algebraic folds (exact): bk shifts every logit of a query by a constant ->
softmax-invariant -> dropped.  bv adds sum(weights)*bv = bv to every attention
output -> (bv @ wo + bo) added on host.
"""

import math
import sys
import types
from contextlib import ExitStack

import numpy as np

# --- shim antenv.axon_hooks so trace=True works under axon -----------------
if "antenv.axon_hooks" not in sys.modules:
    _mod = types.ModuleType("antenv.axon_hooks")
    _hook_box = [None]
    _mod.set_axon_ntff_profile_hook = lambda h: _hook_box.__setitem__(0, h)
    _mod.get_axon_ntff_profile_hook = lambda: _hook_box[0]
    sys.modules["antenv.axon_hooks"] = _mod
    try:
        import antenv

        antenv.axon_hooks = _mod
        from trn_agent_boot.trn_boot import _ntff_profile_via_ctypes

        _mod.set_axon_ntff_profile_hook(
            _ntff_profile_via_ctypes("/opt/axon/libaxon_pjrt.so")
        )
    except Exception:
        pass

import concourse.bass as bass
import concourse.mybir as mybir
import concourse.tile as tile
from concourse.bass_utils import run_bass_kernel_spmd
from concourse.vector_clock import ScopedClock

# --- patch Tile tail drain: this walrus build rejects CTRL insts with >1-2
# sem waits ("Too many sync wait commands").  Split the tail drain's waits
# into one drain per outstanding proc tick.
def _drain_and_barrier_split(self, tick_clock, wait_clock):
    nc = self.nc
    g = ScopedClock({None: tick_clock.global_clock})
    for scope, vc in g.items():
        for proc in range(len(vc)):
            t = vc[proc]
            if t > 0:
                sc = ScopedClock()
                sc.require_at_least(scope, proc, t)
                d = nc.sync.drain()
                wait_clock.add_sem_waits(d.ins, sc)
    nc.all_engine_barrier()
    assert self.sems is not None
    popped = nc._tile_sem_poison_stack.pop()
    assert popped is self._sem_poison
    nc.clear_and_free_semaphores(list(self.sems.allocated().values()))
    nc.all_engine_barrier()


tile.TileContext._drain_and_barrier = _drain_and_barrier_split

# This walrus build tolerates only 1 sync wait per instruction.  Hoist excess
# waits onto preceding EVENT_SEMAPHORE nops (the native wait_ge carrier).
_MAX_WAITS = 1
_orig_lower = tile.TileContext._lower_ordered_insts


def _is_self_wait(inst, w):
    # A ge-wait on the instruction's own engine sem is transitively implied
    # by in-order execution (Tile's vector clock is not transitively minimal).
    if w.wait_mode != "sem-ge-imm" or not w.ant_name:
        return False
    eng = str(inst.engine).split(".")[-1]
    return w.ant_name.startswith(eng + "_")


def _lower_split_waits(self, ordered):
    nc = self.nc
    for bb_name, insts in ordered.items():
        out = []
        for inst in insts:
            si = inst.sync_info
            if si is not None and si.on_wait:
                waits = [w for w in si.on_wait if not _is_self_wait(inst, w)]
                if len(waits) != len(si.on_wait) or len(waits) > _MAX_WAITS:
                    excess, keep = waits[:-_MAX_WAITS], waits[-_MAX_WAITS:]
                    for w in excess:
                        d = mybir.InstEventSemaphore(
                            name=nc.get_next_instruction_name(), ins=[], outs=[]
                        )
                        d.engine = inst.engine
                        d.sync_info = mybir.SyncInfo(on_wait=[w], on_update=[])
                        out.append(d)
                    inst.sync_info = mybir.SyncInfo(
                        on_wait=keep, on_update=list(si.on_update)
                    )
            out.append(inst)
        insts[:] = out
    return _orig_lower(self, ordered)


tile.TileContext._lower_ordered_insts = _lower_split_waits

F32 = mybir.dt.float32
F16 = mybir.dt.float16
B, S, D, H = 2, 2048, 1024, 16
DH = 64
G = 4  # head-groups == cores per batch
CD = D // G  # 256 head dims per core (4 heads)
N_CORES = 8
NEG = -1.0e30
K_PAD_LADDER = (1152, 1536, 2304)  # multiples of MEMC=384
Exp = mybir.ActivationFunctionType.Exp
Ln = mybir.ActivationFunctionType.Ln


def _chunks(total, sz):
    out = []
    off = 0
    while off < total:
        c = min(sz, total - off)
        out.append((off, c))
        off += c
    return out


MEMC = 384  # memT column-chunk width (k_pad must be divisible)


def _build(k_pad: int) -> bass.Bass:
    kt_tiles = k_pad // 128
    n_mc = k_pad // MEMC

    nc = bass.Bass()
    # All inputs are host-pre-shuffled so every DMA is a contiguous slab
    # (strided rearranges cost ~2.4us of engine issue time each).
    xT = nc.dram_tensor("xT", [2, 4, 128, 4, 512], F16, kind="ExternalInput")
    memT = nc.dram_tensor("memT", [2, n_mc, 128, 4, MEMC], F16, kind="ExternalInput")
    wq_d = nc.dram_tensor("wq", [128, 8, CD], F16, kind="ExternalInput")
    wk_d = nc.dram_tensor("wk", [128, 8, CD], F16, kind="ExternalInput")
    wv_d = nc.dram_tensor("wv", [128, 8, CD], F16, kind="ExternalInput")
    bq_d = nc.dram_tensor("bq", [128, 2], F32, kind="ExternalInput")
    wo_d = nc.dram_tensor("wo", [128, 2, D], F16, kind="ExternalInput")
    mb_d = nc.dram_tensor("maskb", [128, kt_tiles], F32, kind="ExternalInput")
    out_d = nc.dram_tensor("out", [S, D], F16, kind="ExternalOutput")

    with tile.TileContext(nc) as tc, ExitStack() as ctx:
        consts = ctx.enter_context(tc.tile_pool(name="consts", bufs=1))
        bigin = ctx.enter_context(tc.tile_pool(name="bigin", bufs=1))
        wpool = ctx.enter_context(tc.tile_pool(name="wp", bufs=1))
        qkv = ctx.enter_context(tc.tile_pool(name="qkv", bufs=1))
        ppool = ctx.enter_context(tc.tile_pool(name="pp", bufs=22))
        npool = ctx.enter_context(tc.tile_pool(name="np", bufs=2))
        opool = ctx.enter_context(tc.tile_pool(name="op", bufs=4))
        # PSUM: lt 2x[128,1024] (4 banks) + av 3x[65->128,512] (3) + work 1
        psum = ctx.enter_context(tc.tile_pool(name="ps", bufs=1, space="PSUM"))

        # E2 selector for the 1/s broadcast: bc rows 0:64 <- rcp row 0,
        # rows 64:128 <- rcp row 32.  e2 (+1) for the tail's Exp(-Ln) path,
        # e2n (-1) for the DVE Newton path (which yields -1/s).
        e2 = consts.tile([33, 128], F16, tag="E2")
        nc.vector.memset(e2, 0.0)
        nc.vector.memset(e2[0:1, 0:64], 1.0)
        nc.vector.memset(e2[32:33, 64:128], 1.0)
        e2n = consts.tile([33, 128], F16, tag="E2N")
        nc.vector.memset(e2n, 0.0)
        nc.vector.memset(e2n[0:1, 0:64], -1.0)
        nc.vector.memset(e2n[32:33, 64:128], -1.0)
        # magic constant for the fast-inverse seed: bits 0x7EF477D5
        mgc = consts.tile([33, 512], F32, tag="MGC")
        nc.vector.memset(mgc, 1.6247691722546535e+38)

        bq_sb = consts.tile([128, 2], F32, tag="bq")
        mb_sb = consts.tile([128, kt_tiles], F32, tag="mb")
        # touch Exp+Ln once so the ACT table loads happen during the DMA wait,
        # not in front of the first real exp
        warm = consts.tile([1, 2], F32, tag="warm")
        nc.vector.memset(warm, 1.0)
        nc.scalar.activation(warm[0:1, 0:1], warm[0:1, 1:2], Exp)
        nc.scalar.activation(warm[0:1, 0:1], warm[0:1, 1:2], Ln)
        # spin the PE during the input-DMA wait: the clock ramps with
        # activity (~1.2GHz until a few dozen matmuls retire), so burn the
        # idle window on dummies instead of running the real prologue at
        # half rate.
        wm_ps = psum.tile([128, 512], F32, tag="work", bufs=1, name="wm_ps")
        for _ in range(64):
            nc.tensor.matmul(wm_ps[:, 0:128], e2, e2, start=True, stop=True)

        wk_sb = wpool.tile([128, 8, CD], F16, tag="wk")
        wv_sb = wpool.tile([128, 8, CD], F16, tag="wv")
        wq_sb = wpool.tile([128, 8, CD], F16, tag="wq")
        wo_sb = wpool.tile([128, 2, D], F16, tag="wo")
        memc = [
            [
                bigin.tile([128, 4, MEMC], F16, tag=f"memc{h}_{c}", name=f"memc{h}_{c}")
                for c in range(n_mc)
            ]
            for h in range(2)
        ]
        xc = [
            [
                bigin.tile([128, 4, 512], F16, tag=f"xc{h}_{q}", name=f"xc{h}_{q}")
                for q in range(4)
            ]
            for h in range(2)
        ]

        # --- input DMA (all contiguous slabs), ordered for the EARLY-START
        # critical path: the attention unit stream begins as soon as
        # wk + wq + xc[*][0] + memc[*][0] + mb/bq have landed (~15us), so
        # those lead both queues; memT/wv/late xc chunks follow.
        nc.scalar.dma_start(out=mb_sb, in_=mb_d[:, :])
        nc.scalar.dma_start(out=bq_sb, in_=bq_d[:, :])
        nc.sync.dma_start(out=wk_sb[:, 0:4, :], in_=wk_d[:, 0:4, :])
        nc.scalar.dma_start(out=wk_sb[:, 4:8, :], in_=wk_d[:, 4:8, :])
        nc.sync.dma_start(out=xc[0][0], in_=xT[0, 0])
        nc.scalar.dma_start(out=xc[1][0], in_=xT[1, 0])
        nc.sync.dma_start(out=wq_sb[:, 0:4, :], in_=wq_d[:, 0:4, :])
        nc.scalar.dma_start(out=wq_sb[:, 4:8, :], in_=wq_d[:, 4:8, :])
        for c in range(n_mc):
            nc.sync.dma_start(out=memc[0][c], in_=memT[0, c])
            nc.scalar.dma_start(out=memc[1][c], in_=memT[1, c])
        nc.scalar.dma_start(out=wv_sb, in_=wv_d[:, :, :])
        nc.sync.dma_start(out=xc[0][1], in_=xT[0, 1])
        nc.scalar.dma_start(out=xc[1][1], in_=xT[1, 1])
        nc.sync.dma_start(out=wo_sb[:, 0, :], in_=wo_d[:, 0, :])
        nc.scalar.dma_start(out=wo_sb[:, 1, :], in_=wo_d[:, 1, :])
        for q in range(2, 4):
            nc.sync.dma_start(out=xc[0][q], in_=xT[0, q])
            nc.scalar.dma_start(out=xc[1][q], in_=xT[1, q])

        # --- K/V/Q0 projections (psum via the "av" 3-slot rotation) --------
        # query chunks: last 512 split in two so the drain tail is half-sized
        QW = (512, 512, 512, 512)
        QOFF = (0, 512, 1024, 1536)
        NQ = len(QW)
        QT = [
            [
                qkv.tile([128, QW[q]], F16, tag=f"QT{p}_{q}", name=f"QT{p}_{q}")
                for q in range(NQ)
            ]
            for p in range(2)
        ]
        KT = [qkv.tile([128, k_pad], F16, tag=f"KT{p}", name=f"KT{p}") for p in range(2)]
        # V with a ones column per head: [128 keys, head, 64 dims + 1]
        V = [
            qkv.tile([128, 4, 65], F16, tag=f"V{kt}", name=f"V{kt}")
            for kt in range(kt_tiles)
        ]
        for kt in range(kt_tiles):
            nc.vector.memset(V[kt][:, :, 64:65], 1.0)

        def k_chunk(pair, c, tag, bufs):
            cs = slice(128 * pair, 128 * pair + 128)
            k_ps = psum.tile([128, MEMC], F32, tag=tag, bufs=bufs, name="k_ps")
            for dt in range(8):
                nc.tensor.matmul(
                    k_ps,
                    wk_sb[:, dt, cs],
                    memc[dt // 4][c][:, dt % 4, :],
                    start=(dt == 0),
                    stop=(dt == 7),
                )
            nc.vector.tensor_copy(KT[pair][:, c * MEMC : (c + 1) * MEMC], k_ps)

        def v_chunk(kt, tag, bufs):
            c, coff = divmod(kt * 128, MEMC)
            v_ps = psum.tile([128, 4, 64], F32, tag=tag, bufs=bufs, name="v_ps")
            for dt in range(8):
                nc.tensor.matmul(
                    v_ps,
                    memc[dt // 4][c][:, dt % 4, coff : coff + 128],
                    wv_sb[:, dt, :],
                    start=(dt == 0),
                    stop=(dt == 7),
                )
            nc.vector.tensor_copy(V[kt][:, :, 0:64], v_ps)

        def q0_chunk(pair, tag, bufs):
            cs = slice(128 * pair, 128 * pair + 128)
            q_ps = psum.tile([128, 512], F32, tag=tag, bufs=bufs, name="q_ps")
            for dt in range(8):
                nc.tensor.matmul(
                    q_ps,
                    wq_sb[:, dt, cs],
                    xc[dt // 4][0][:, dt % 4, :],
                    start=(dt == 0),
                    stop=(dt == 7),
                )
            nc.vector.tensor_scalar_add(QT[pair][0], q_ps, bq_sb[:, pair : pair + 1])

        # minimal prologue: just what unit (qc0, pair0, kt0) needs.  The
        # remaining projection groups are interleaved one-per-unit into the
        # early attention stream (their psum "av" rotation is free because AV
        # flushes are deferred until the projections complete).  Group order
        # follows DMA arrival so a stalled group never blocks a ready QK.
        q0_chunk(0, "av", 3)  # q0 inputs land ~2us before memc: keep PE fed
        k_chunk(0, 0, "av", 3)
        proj = [
            lambda: k_chunk(1, 0, "av", 3),
            lambda: q0_chunk(1, "av", 3),
        ]
        for c in range(1, n_mc):
            proj.append(lambda c=c: k_chunk(0, c, "av", 3))
            proj.append(lambda c=c: k_chunk(1, c, "av", 3))
        proj.extend(
            lambda kt=kt: v_chunk(kt, "av", 3) for kt in range(kt_tiles)
        )

        def q_stream_gen(qqc):
            # one q-chunk through the "work" psum slot; fine-grained 4-dt steps
            w, off = QW[qqc], QOFF[qqc]
            blk, boff = divmod(off, 512)
            for pair in range(2):
                cs = slice(128 * pair, 128 * pair + 128)
                q_ps = psum.tile([128, 512], F32, tag="work", bufs=1, name="q_ps")
                for dt in range(8):
                    nc.tensor.matmul(
                        q_ps[:, 0:w],
                        wq_sb[:, dt, cs],
                        xc[dt // 4][blk][:, dt % 4, boff : boff + w],
                        start=(dt == 0),
                        stop=(dt == 7),
                    )
                    if dt == 3:
                        yield
                nc.vector.tensor_scalar_add(
                    QT[pair][qqc], q_ps[:, 0:w], bq_sb[:, pair : pair + 1]
                )
                yield

        # --- attention: flat software-pipelined stream, pair-major ---------
        outT = [
            [
                qkv.tile([128, QW[q]], F16, tag=f"oT{p}_{q}", name=f"oT{p}_{q}")
                for q in range(NQ)
            ]
            for p in range(2)
        ]

        st_av = {}  # (qc, pair) -> [avA, avB] psum tiles [65, w]
        epi_count = [0]
        completed = []  # (qc, pair) whose final AV flush has been emitted
        nflush = {}  # (qc, pair) -> AV flushes emitted so far
        group_order = []  # (qc, pair) in first-push order
        group_seen = set()

        def flush(pend):
            # AV accumulation is order-independent over kt: start/stop flags
            # follow the per-pair flush COUNT, not the kt index.
            qc, pair, kt, p_t = pend
            w = QW[qc]
            n = nflush.get((qc, pair), 0)
            nflush[(qc, pair)] = n + 1
            av = st_av.setdefault((qc, pair), [None, None])
            for a in range(2):
                if av[a] is None:
                    av[a] = psum.tile(
                        [65, 512], F32, tag="av", bufs=3, name=f"av{pair}_{a}"
                    )[:, 0:w]
                nc.tensor.matmul(
                    av[a],
                    V[kt][:, 2 * pair + a, :],
                    p_t[:, a, :],
                    start=(n == 0),
                    stop=(n == kt_tiles - 1),
                )
            if n == kt_tiles - 1:
                completed.append((qc, pair))

        def drain_one(pending):
            # Flush only the oldest still-accumulating pair: at most two pairs
            # may hold av psum banks at once (3-slot rotation).
            g = next(
                (k for k in group_order if nflush.get(k, 0) < kt_tiles), None
            )
            idx = next(
                (i for i, p in enumerate(pending) if (p[0], p[1]) == g), None
            )
            if idx is None:
                return False
            flush(pending.pop(idx))
            return True

        def pair_epilogue(qc, pair):
            """1/s via Ln+Exp, E2-broadcast, fused normalize; out-proj on pair 1."""
            w, off = QW[qc], QOFF[qc]
            tail = qc == NQ - 1 and pair == 1  # nothing left to overlap with
            avA, avB = st_av[(qc, pair)]
            g = npool.tile([33, 512], F32, tag="g", bufs=4)
            if epi_count[0] < 4:
                nc.vector.memset(g, 1.0)  # keep Ln input positive in slack rows
            epi_count[0] += 1

            def outproj(sl, wtag, wbufs, ev_alt, cts=(0, 1)):
                o_sb = opool.tile([128, 1024], F16, tag="osb", name="o_sb")
                for nch in range(2):
                    o_ps = psum.tile(
                        [128, 512], F32,
                        tag=(wtag if nch % 2 else "work"),
                        bufs=(wbufs if nch % 2 else 1),
                        name="o_ps",
                    )
                    for j, ct in enumerate(cts):
                        nc.tensor.matmul(
                            o_ps,
                            outT[ct][qc][:, sl * 128 : (sl + 1) * 128],
                            wo_sb[:, ct, nch * 512 : (nch + 1) * 512],
                            start=(j == 0),
                            stop=(j == len(cts) - 1),
                        )
                    if ev_alt and nch % 2:
                        nc.scalar.copy(o_sb[:, nch * 512 : (nch + 1) * 512], o_ps)
                    else:
                        nc.vector.tensor_copy(
                            o_sb[:, nch * 512 : (nch + 1) * 512], o_ps
                        )
                return o_sb

            if tail:
                # final 256-q chunk: short serial chain, scalar engine takes
                # the gathers + odd evacs since it is idle here
                nc.scalar.copy(g[0:1, 0:w], avA[64:65, :])
                nc.scalar.copy(g[32:33, 0:w], avB[64:65, :])
                lns = npool.tile([33, w], F32, tag="lns", bufs=4)
                nc.scalar.activation(lns, g[:, 0:w], Ln)
                rcp16 = npool.tile([33, w], F16, tag="rcp16", bufs=4)
                nc.scalar.activation(rcp16, lns, Exp, scale=-1.0)
                bc_ps = psum.tile([128, w], F32, tag="work", bufs=1, name="bc_ps")
                nc.tensor.matmul(bc_ps, e2, rcp16, start=True, stop=True)
                bc_sb = npool.tile([128, w], F32, tag="bc_sb", bufs=4)
                nc.vector.tensor_copy(bc_sb, bc_ps)
                nc.vector.tensor_tensor(
                    outT[pair][qc][0:64, :], avA[0:64, :], bc_sb[0:64, :],
                    op=mybir.AluOpType.mult,
                )
                nc.vector.tensor_tensor(
                    outT[pair][qc][64:128, :], avB[0:64, :], bc_sb[64:128, :],
                    op=mybir.AluOpType.mult,
                )
                for sl in range(w // 128):
                    o_sb = outproj(sl, "lt", 2, ev_alt=True)
                    st = off // 128 + sl
                    eng = nc.scalar if sl % 2 else nc.sync
                    eng.dma_start(out=out_d[st * 128 : (st + 1) * 128, :], in_=o_sb)
                del st_av[(qc, pair)]
                return

            nc.vector.tensor_copy(g[0:1, 0:w], avA[64:65, :])
            nc.vector.tensor_copy(g[32:33, 0:w], avB[64:65, :])
            yield
            # 1/s on the DVE via fast-inverse seed + 2 Newton steps, keeping
            # the scalar engine free for the exp stream.  x0 = bitcast(MAGIC -
            # bitcast(s)); xk alternates sign: the final value is -1/s,
            # absorbed by the negated e2n selector.
            U32 = mybir.dt.uint32
            x0 = npool.tile([33, w], F32, tag="lns", bufs=4)
            nc.vector.tensor_tensor(
                x0.bitcast(U32), mgc[:, 0:w].bitcast(U32), g[:, 0:w].bitcast(U32),
                op=mybir.AluOpType.subtract,
            )
            t1 = npool.tile([33, w], F32, tag="nt", bufs=2)
            nc.vector.tensor_tensor(t1, g[:, 0:w], x0, op=mybir.AluOpType.mult)
            yield
            rcp16 = npool.tile([33, w], F16, tag="rcp16", bufs=4)
            nc.vector.scalar_tensor_tensor(
                rcp16, t1, 2.0, x0,
                op0=mybir.AluOpType.subtract, op1=mybir.AluOpType.mult,
            )
            yield
            bc_ps = psum.tile([128, w], F32, tag="work", bufs=1, name="bc_ps")
            nc.tensor.matmul(bc_ps, e2n, rcp16, start=True, stop=True)
            bc_sb = npool.tile([128, w], F32, tag="bc_sb", bufs=4)
            nc.vector.tensor_copy(bc_sb, bc_ps)
            yield
            nc.vector.tensor_tensor(
                outT[pair][qc][0:64, :], avA[0:64, :], bc_sb[0:64, :],
                op=mybir.AluOpType.mult,
            )
            nc.vector.tensor_tensor(
                outT[pair][qc][64:128, :], avB[0:64, :], bc_sb[64:128, :],
                op=mybir.AluOpType.mult,
            )
            del st_av[(qc, pair)]
            yield
            if pair == 0:
                return
            for sl in range(w // 128):
                o_sb = outproj(sl, "work", 1, ev_alt=False)
                yield
                st = off // 128 + sl
                nc.sync.dma_start(out=out_d[st * 128 : (st + 1) * 128, :], in_=o_sb)

        # qc0's units run during the projection interleave, so they follow
        # memT chunk arrival (kt blocks of 3, both pairs per block); later
        # chunks are pair-major as before.
        units = [
            (0, pair, 3 * blk + k)
            for blk in range(n_mc)
            for pair in range(2)
            for k in range(3)
        ]
        units += [
            (qc, pair, kt)
            for qc in range(1, NQ)
            for pair in range(2)
            for kt in range(kt_tiles)
        ]

        def _step2(gens):
            # two steps per unit: halves epilogue wall-latency so av psum
            # banks recycle sooner at pair transitions
            out = []
            for g_ in gens:
                if next(g_, StopIteration) is StopIteration:
                    continue
                if next(g_, StopIteration) is StopIteration:
                    continue
                out.append(g_)
            return out

        prio = []
        # q-chunk streams gated into the PE-slack zone (chunk qi's QT is
        # needed at unit qi*18); chunk 1 waits for its xc DMA (~unit 3).
        qgens = [
            (q_stream_gen(1), 8),
            (q_stream_gen(2), 27),
            (q_stream_gen(3), 45),
        ]
        pending = []
        for u, (qc, pair, kt) in enumerate(units):
            # QK first: the tensor queue head must be the op that feeds the
            # exp stream, never something that waits on it.
            w = QW[qc]
            ks = slice(kt * 128, (kt + 1) * 128)
            # psum accumulation groups must stay bank-aligned: keep the two
            # QK halves in separate 2KB banks even for 256-wide chunks
            lt_full = psum.tile([128, 2, 512], F32, tag="lt", bufs=2, name="lt")
            lt = lt_full[:, :, 0:w]
            nc.tensor.matmul(
                lt[:, 0, :], KT[pair][0:64, ks], QT[pair][qc][0:64, :],
                start=True, stop=True,
            )
            nc.tensor.matmul(
                lt[:, 1, :], KT[pair][64:128, ks], QT[pair][qc][64:128, :],
                start=True, stop=True,
            )
            p_full = ppool.tile([128, 2, 512], F16, tag="p")
            p_t = p_full[:, :, 0:w]
            nc.scalar.activation(
                p_t, lt, Exp, bias=mb_sb[:, kt : kt + 1], scale=0.125
            )
            if (qc, pair) not in group_seen:
                group_seen.add((qc, pair))
                group_order.append((qc, pair))
            pending.append((qc, pair, kt, p_t))
            if proj:
                # one deferred projection group per unit; AV flushes wait
                # (ppool is deep enough to hold the p_t backlog)
                proj.pop(0)()
            else:
                # drain the flush backlog at up to 2/unit, then steady-state
                # with a 4-unit lag (the lag keeps the in-order tensor queue
                # from stalling on an exp still in flight)
                drains = 0
                while len(pending) > 4 and drains < 2:
                    if not drain_one(pending):
                        break
                    drains += 1
            for key in completed:
                prio.append(pair_epilogue(*key))
            completed.clear()
            while qgens and qgens[0][1] <= u:
                if next(qgens[0][0], StopIteration) is StopIteration:
                    qgens.pop(0)
                break
            prio = _step2(prio)

        while pending:
            assert drain_one(pending)
            for key in completed:
                prio.append(pair_epilogue(*key))
            completed.clear()
            prio = _step2(prio)
        for gen, _ in qgens:
            for _ in gen:
                pass
        for gen in prio:
            for _ in gen:
                pass

    return nc


_PROG_CACHE: dict[int, bass.Bass] = {}


def _get_prog(k_pad: int) -> bass.Bass:
    if k_pad not in _PROG_CACHE:
        _PROG_CACHE[k_pad] = _build(k_pad)
    return _PROG_CACHE[k_pad]


def _shuf_w(w):
    # [1024, CD] -> [128, 8, CD]: sbuf partition p holds rows 128*t + p
    return np.ascontiguousarray(w.astype(np.float16).reshape(8, 128, -1).transpose(1, 0, 2))


def _shuf_chunks(tT, csz):
    # [D, N] -> [2, N/csz, 128, 4, csz] contiguous DMA slabs per (half, chunk)
    d, n = tT.shape
    a = tT.reshape(2, 4, 128, n // csz, csz)  # [half, t, p, chunk, csz]
    return np.ascontiguousarray(a.transpose(0, 3, 2, 1, 4))


def _prep_inputs(x, memory, mask, wq, bq, wk, wv, k_pad):
    """Build the 8 per-core input maps."""
    kt_tiles = k_pad // 128
    in_maps = []
    per_batch = []
    for b in range(B):
        idx = np.flatnonzero(~mask[b])
        n = len(idx)
        assert n <= k_pad
        mem_g = np.zeros((k_pad, D), np.float16)
        mem_g[:n] = memory[b][idx].astype(np.float16)
        memT_b = _shuf_chunks(np.ascontiguousarray(mem_g.T), MEMC)
        xT_b = _shuf_chunks(np.ascontiguousarray(x[b].astype(np.float16).T), 512)
        mbias = np.zeros(k_pad, np.float32)
        mbias[n:] = NEG
        mb_b = np.ascontiguousarray(mbias.reshape(kt_tiles, 128).T)
        per_batch.append((xT_b, memT_b, mb_b, idx))
    for c in range(N_CORES):
        b, g = divmod(c, G)
        xT_b, memT_b, mb_b, _ = per_batch[b]
        cs = slice(g * CD, (g + 1) * CD)
        in_maps.append(
            {
                "xT": xT_b,
                "memT": memT_b,
                "wq": _shuf_w(wq[:, cs]),
                "wk": _shuf_w(wk[:, cs]),
                "wv": _shuf_w(wv[:, cs]),
                "bq": np.ascontiguousarray(bq[cs].reshape(2, 128).T.astype(np.float32)),
                "wo": None,  # filled by caller (needs wo)
                "maskb": mb_b,
            }
        )
    return in_maps, per_batch


def kernel(x, memory, mask, wq, bq, wk, bk, wv, bv, wo, bo, _trace=False):
    x = np.asarray(x, np.float32)
    memory = np.asarray(memory, np.float32)
    mask = np.asarray(mask).astype(bool)
    wq = np.asarray(wq, np.float32)
    bq = np.asarray(bq, np.float32)
    wk = np.asarray(wk, np.float32)
    wv = np.asarray(wv, np.float32)
    bv = np.asarray(bv, np.float32)
    wo = np.asarray(wo, np.float32)
    bo = np.asarray(bo, np.float32)

    nmax = max(int((~mask[b]).sum()) for b in range(B))
    k_pad = next(k for k in K_PAD_LADDER if k >= nmax)
    prog = _get_prog(k_pad)

    in_maps, _ = _prep_inputs(x, memory, mask, wq, bq, wk, wv, k_pad)
    for c in range(N_CORES):
        g = c % G
        in_maps[c]["wo"] = np.ascontiguousarray(
            wo[g * CD : (g + 1) * CD, :].astype(np.float16).reshape(2, 128, D).transpose(1, 0, 2)
        )

    res = run_bass_kernel_spmd(prog, in_maps, list(range(N_CORES)), trace=_trace)
    outs = [res.results[c]["out"] for c in range(N_CORES)]
    final = np.empty((B, S, D), np.float32)
    tail = bo + bv @ wo
    for b in range(B):
        final[b] = outs[G * b].astype(np.float32)
        for g in range(1, G):
            final[b] += outs[G * b + g].astype(np.float32)
        final[b] += tail[None, :]
    if _trace:
        kernel.last_exec_time_ns = res.exec_time_ns
    return final



# revision 45
# speedup vs baseline: 1.0378x; 1.0307x over previous
"""Multi-head cross-attention (B=2, S=2048, D=1024, H=16) on 8 trn2 cores.

Sharding: core c -> (batch b = c//4, head-group g = c%4, 4 heads per group).
Tensor-parallel heads: wq/wk/wv column-sliced, wo row-sliced; partial outputs
summed on host.  Key-mask compaction on host: only unmasked keys are shipped
(padded to K_PAD), since masked keys contribute exactly zero after softmax.

v2 vs baseline:
- softmax sums folded into the AV matmul via a ones column appended to V
  (lhsT [128, 65] -> av psum [65, 512], row 64 = sums); the 144 separate
  ones-matmul streams are gone (-31us of PE).
- attention units ordered pair-major (qc, pair, kt) so only 2+1 av psum
  banks are needed, freeing a bank for the q-proj/out-proj "work" slot.
- normalization fused into psum evacuation: outT = av_psum * bc in one DVE
  tensor_tensor (probed: mixed partition bases legal when in0 is PSUM).
- fp16 output DMA (half the out traffic); host accumulates partials in f32.

v3 (this file) vs v2 — measured ~154us -> ~150us:
- 1/sums moved OFF the saturated ACT engine (its exp stream is the phase
  co-bottleneck at ~1us/unit): fast-inverse bit trick + one Newton step on
  the DVE (3 ops on [33, 512]); the Newton sign flip is absorbed by a
  negated broadcast selector e2n.  The serial tail keeps Ln/Exp on ACT
  (idle there).  Max 1/s error ~6e-3 -> rel err 1.9e-3 (gate 2e-2).
- EARLY-START schedule: only k(pair0, c0) + q0(pair0) run as prologue;
  the other 15 projection groups are interleaved one-per-unit into the
  attention stream (ordered by DMA arrival), so the first exp fires at
  ~18us instead of ~48us.  qc0's units follow memT chunk arrival
  (kt-blocks of 3, both pairs per block).  AV flushes are DEFERRED while
  projections own the "av" psum rotation (ppool holds the p_t backlog),
  then drain 2/unit grouped by pair -- AV accumulation order over kt is
  irrelevant, only the per-pair first/last flags matter.
- input DMA reordered for the early-start critical path (mb/bq/wk/xc0/wq
  lead, then memT, wv, late xc chunks); wq/wo split across both queues.
- PE p-state warmup: the clock sits at 1.2GHz until a few dozen matmuls
  retire, so 64 dummy matmuls burn the initial DMA window and the real
  prologue starts at full rate.
- epilogue generators advance 2 steps/unit so av banks recycle faster at
  pair transitions.

Known-remaining (measured, resisted 3 scheduling attempts each): ~8us of
qc-boundary ACT stalls (out-proj PE surplus), ~10us endgame at half clock
(p-state drops in the serial tail; dummy-matmul keep-alive backfired by
serializing into the dependency chain), ~7us NEFF semaphore-reset
ceremony (compiler-inserted, one inst per ~256 sems).

Algebraic folds (exact): bk shifts every logit of a query by a constant ->
softmax-invariant -> dropped.  bv adds sum(weights)*bv = bv to every attention
output -> (bv @ wo + bo) added on host.
"""

import math
import sys
import types
from contextlib import ExitStack

import numpy as np

# --- shim antenv.axon_hooks so trace=True works under axon -----------------
if "antenv.axon_hooks" not in sys.modules:
    _mod = types.ModuleType("antenv.axon_hooks")
    _hook_box = [None]
    _mod.set_axon_ntff_profile_hook = lambda h: _hook_box.__setitem__(0, h)
    _mod.get_axon_ntff_profile_hook = lambda: _hook_box[0]
    sys.modules["antenv.axon_hooks"] = _mod
    try:
        import antenv

        antenv.axon_hooks = _mod
        from trn_agent_boot.trn_boot import _ntff_profile_via_ctypes

        _mod.set_axon_ntff_profile_hook(
            _ntff_profile_via_ctypes("/opt/axon/libaxon_pjrt.so")
        )
    except Exception:
        pass

import concourse.bass as bass
import concourse.mybir as mybir
import concourse.tile as tile
from concourse.bass_utils import run_bass_kernel_spmd
from concourse.vector_clock import ScopedClock

# --- patch Tile tail drain: this walrus build rejects CTRL insts with >1-2
# sem waits ("Too many sync wait commands").  Split the tail drain's waits
# into one drain per outstanding proc tick.
def _drain_and_barrier_split(self, tick_clock, wait_clock):
    nc = self.nc
    g = ScopedClock({None: tick_clock.global_clock})
    for scope, vc in g.items():
        for proc in range(len(vc)):
            t = vc[proc]
            if t > 0:
                sc = ScopedClock()
                sc.require_at_least(scope, proc, t)
                d = nc.sync.drain()
                wait_clock.add_sem_waits(d.ins, sc)
    nc.all_engine_barrier()
    assert self.sems is not None
    popped = nc._tile_sem_poison_stack.pop()
    assert popped is self._sem_poison
    nc.clear_and_free_semaphores(list(self.sems.allocated().values()))
    nc.all_engine_barrier()


tile.TileContext._drain_and_barrier = _drain_and_barrier_split

# This walrus build tolerates only 1 sync wait per instruction.  Hoist excess
# waits onto preceding EVENT_SEMAPHORE nops (the native wait_ge carrier).
_MAX_WAITS = 1
_orig_lower = tile.TileContext._lower_ordered_insts


def _is_self_wait(inst, w):
    # A ge-wait on the instruction's own engine sem is transitively implied
    # by in-order execution (Tile's vector clock is not transitively minimal).
    if w.wait_mode != "sem-ge-imm" or not w.ant_name:
        return False
    eng = str(inst.engine).split(".")[-1]
    return w.ant_name.startswith(eng + "_")


def _lower_split_waits(self, ordered):
    nc = self.nc
    for bb_name, insts in ordered.items():
        out = []
        for inst in insts:
            si = inst.sync_info
            if si is not None and si.on_wait:
                waits = [w for w in si.on_wait if not _is_self_wait(inst, w)]
                if len(waits) != len(si.on_wait) or len(waits) > _MAX_WAITS:
                    excess, keep = waits[:-_MAX_WAITS], waits[-_MAX_WAITS:]
                    for w in excess:
                        d = mybir.InstEventSemaphore(
                            name=nc.get_next_instruction_name(), ins=[], outs=[]
                        )
                        d.engine = inst.engine
                        d.sync_info = mybir.SyncInfo(on_wait=[w], on_update=[])
                        out.append(d)
                    inst.sync_info = mybir.SyncInfo(
                        on_wait=keep, on_update=list(si.on_update)
                    )
            out.append(inst)
        insts[:] = out
    return _orig_lower(self, ordered)


tile.TileContext._lower_ordered_insts = _lower_split_waits

F32 = mybir.dt.float32
F16 = mybir.dt.float16
B, S, D, H = 2, 2048, 1024, 16
DH = 64
G = 4  # head-groups == cores per batch
CD = D // G  # 256 head dims per core (4 heads)
N_CORES = 8
NEG = -1.0e30
K_PAD_LADDER = (1152, 1536, 2304)  # multiples of MEMC=384
Exp = mybir.ActivationFunctionType.Exp
Ln = mybir.ActivationFunctionType.Ln


def _chunks(total, sz):
    out = []
    off = 0
    while off < total:
        c = min(sz, total - off)
        out.append((off, c))
        off += c
    return out


MEMC = 384  # memT column-chunk width (k_pad must be divisible)


def _build(k_pad: int) -> bass.Bass:
    kt_tiles = k_pad // 128
    n_mc = k_pad // MEMC

    nc = bass.Bass()
    # All inputs are host-pre-shuffled so every DMA is a contiguous slab
    # (strided rearranges cost ~2.4us of engine issue time each).
    xT = nc.dram_tensor("xT", [2, 4, 128, 4, 512], F16, kind="ExternalInput")
    memT = nc.dram_tensor("memT", [2, n_mc, 128, 4, MEMC], F16, kind="ExternalInput")
    wq_d = nc.dram_tensor("wq", [128, 8, CD], F16, kind="ExternalInput")
    wk_d = nc.dram_tensor("wk", [128, 8, CD], F16, kind="ExternalInput")
    wv_d = nc.dram_tensor("wv", [128, 8, CD], F16, kind="ExternalInput")
    bq_d = nc.dram_tensor("bq", [128, 2], F32, kind="ExternalInput")
    wo_d = nc.dram_tensor("wo", [128, 2, D], F16, kind="ExternalInput")
    mb_d = nc.dram_tensor("maskb", [128, kt_tiles], F32, kind="ExternalInput")
    out_d = nc.dram_tensor("out", [S, D], F16, kind="ExternalOutput")

    with tile.TileContext(nc) as tc, ExitStack() as ctx:
        consts = ctx.enter_context(tc.tile_pool(name="consts", bufs=1))
        bigin = ctx.enter_context(tc.tile_pool(name="bigin", bufs=1))
        wpool = ctx.enter_context(tc.tile_pool(name="wp", bufs=1))
        qkv = ctx.enter_context(tc.tile_pool(name="qkv", bufs=1))
        ppool = ctx.enter_context(tc.tile_pool(name="pp", bufs=22))
        npool = ctx.enter_context(tc.tile_pool(name="np", bufs=2))
        opool = ctx.enter_context(tc.tile_pool(name="op", bufs=4))
        # PSUM: lt 2x[128,1024] (4 banks) + av 3x[65->128,512] (3) + work 1
        psum = ctx.enter_context(tc.tile_pool(name="ps", bufs=1, space="PSUM"))

        # E2 selector for the 1/s broadcast: bc rows 0:64 <- rcp row 0,
        # rows 64:128 <- rcp row 32.  e2 (+1) for the tail's Exp(-Ln) path,
        # e2n (-1) for the DVE Newton path (which yields -1/s).
        e2 = consts.tile([33, 128], F16, tag="E2")
        nc.vector.memset(e2, 0.0)
        nc.vector.memset(e2[0:1, 0:64], 1.0)
        nc.vector.memset(e2[32:33, 64:128], 1.0)
        e2n = consts.tile([33, 128], F16, tag="E2N")
        nc.vector.memset(e2n, 0.0)
        nc.vector.memset(e2n[0:1, 0:64], -1.0)
        nc.vector.memset(e2n[32:33, 64:128], -1.0)
        # magic constant for the fast-inverse seed: bits 0x7EF477D5
        mgc = consts.tile([33, 512], F32, tag="MGC")
        nc.vector.memset(mgc, 1.6247691722546535e+38)

        bq_sb = consts.tile([128, 2], F32, tag="bq")
        mb_sb = consts.tile([128, kt_tiles], F32, tag="mb")
        # touch Exp+Ln once so the ACT table loads happen during the DMA wait,
        # not in front of the first real exp
        warm = consts.tile([1, 2], F32, tag="warm")
        nc.vector.memset(warm, 1.0)
        nc.scalar.activation(warm[0:1, 0:1], warm[0:1, 1:2], Exp)
        nc.scalar.activation(warm[0:1, 0:1], warm[0:1, 1:2], Ln)
        # spin the PE during the input-DMA wait: the clock ramps with
        # activity (~1.2GHz until a few dozen matmuls retire), so burn the
        # idle window on dummies instead of running the real prologue at
        # half rate.
        wm_ps = psum.tile([128, 512], F32, tag="work", bufs=1, name="wm_ps")
        for _ in range(64):
            nc.tensor.matmul(wm_ps[:, 0:128], e2, e2, start=True, stop=True)

        wk_sb = wpool.tile([128, 8, CD], F16, tag="wk")
        wv_sb = wpool.tile([128, 8, CD], F16, tag="wv")
        wq_sb = wpool.tile([128, 8, CD], F16, tag="wq")
        wo_sb = wpool.tile([128, 2, D], F16, tag="wo")
        memc = [
            [
                bigin.tile([128, 4, MEMC], F16, tag=f"memc{h}_{c}", name=f"memc{h}_{c}")
                for c in range(n_mc)
            ]
            for h in range(2)
        ]
        xc = [
            [
                bigin.tile([128, 4, 512], F16, tag=f"xc{h}_{q}", name=f"xc{h}_{q}")
                for q in range(4)
            ]
            for h in range(2)
        ]

        # --- input DMA (all contiguous slabs), ordered for the EARLY-START
        # critical path: the attention unit stream begins as soon as
        # wk + wq + xc[*][0] + memc[*][0] + mb/bq have landed (~15us), so
        # those lead both queues; memT/wv/late xc chunks follow.
        nc.scalar.dma_start(out=mb_sb, in_=mb_d[:, :])
        nc.scalar.dma_start(out=bq_sb, in_=bq_d[:, :])
        nc.sync.dma_start(out=wk_sb[:, 0:4, :], in_=wk_d[:, 0:4, :])
        nc.scalar.dma_start(out=wk_sb[:, 4:8, :], in_=wk_d[:, 4:8, :])
        nc.sync.dma_start(out=xc[0][0], in_=xT[0, 0])
        nc.scalar.dma_start(out=xc[1][0], in_=xT[1, 0])
        nc.sync.dma_start(out=wq_sb[:, 0:4, :], in_=wq_d[:, 0:4, :])
        nc.scalar.dma_start(out=wq_sb[:, 4:8, :], in_=wq_d[:, 4:8, :])
        for c in range(n_mc):
            nc.sync.dma_start(out=memc[0][c], in_=memT[0, c])
            nc.scalar.dma_start(out=memc[1][c], in_=memT[1, c])
        nc.scalar.dma_start(out=wv_sb, in_=wv_d[:, :, :])
        nc.sync.dma_start(out=xc[0][1], in_=xT[0, 1])
        nc.scalar.dma_start(out=xc[1][1], in_=xT[1, 1])
        nc.sync.dma_start(out=wo_sb[:, 0, :], in_=wo_d[:, 0, :])
        nc.scalar.dma_start(out=wo_sb[:, 1, :], in_=wo_d[:, 1, :])
        for q in range(2, 4):
            nc.sync.dma_start(out=xc[0][q], in_=xT[0, q])
            nc.scalar.dma_start(out=xc[1][q], in_=xT[1, q])

        # --- K/V/Q0 projections (psum via the "av" 3-slot rotation) --------
        # query chunks: last 512 split in two so the drain tail is half-sized
        QW = (512, 512, 512, 512)
        QOFF = (0, 512, 1024, 1536)
        NQ = len(QW)
        QT = [
            [
                qkv.tile([128, QW[q]], F16, tag=f"QT{p}_{q}", name=f"QT{p}_{q}")
                for q in range(NQ)
            ]
            for p in range(2)
        ]
        KT = [qkv.tile([128, k_pad], F16, tag=f"KT{p}", name=f"KT{p}") for p in range(2)]
        # V with a ones column per head: [128 keys, head, 64 dims + 1]
        V = [
            qkv.tile([128, 4, 65], F16, tag=f"V{kt}", name=f"V{kt}")
            for kt in range(kt_tiles)
        ]
        for kt in range(kt_tiles):
            nc.vector.memset(V[kt][:, :, 64:65], 1.0)

        def k_chunk(pair, c, tag, bufs):
            cs = slice(128 * pair, 128 * pair + 128)
            k_ps = psum.tile([128, MEMC], F32, tag=tag, bufs=bufs, name="k_ps")
            for dt in range(8):
                nc.tensor.matmul(
                    k_ps,
                    wk_sb[:, dt, cs],
                    memc[dt // 4][c][:, dt % 4, :],
                    start=(dt == 0),
                    stop=(dt == 7),
                )
            nc.vector.tensor_copy(KT[pair][:, c * MEMC : (c + 1) * MEMC], k_ps)

        def v_chunk(kt, tag, bufs):
            c, coff = divmod(kt * 128, MEMC)
            v_ps = psum.tile([128, 4, 64], F32, tag=tag, bufs=bufs, name="v_ps")
            for dt in range(8):
                nc.tensor.matmul(
                    v_ps,
                    memc[dt // 4][c][:, dt % 4, coff : coff + 128],
                    wv_sb[:, dt, :],
                    start=(dt == 0),
                    stop=(dt == 7),
                )
            nc.vector.tensor_copy(V[kt][:, :, 0:64], v_ps)

        def q0_chunk(pair, tag, bufs):
            cs = slice(128 * pair, 128 * pair + 128)
            q_ps = psum.tile([128, 512], F32, tag=tag, bufs=bufs, name="q_ps")
            for dt in range(8):
                nc.tensor.matmul(
                    q_ps,
                    wq_sb[:, dt, cs],
                    xc[dt // 4][0][:, dt % 4, :],
                    start=(dt == 0),
                    stop=(dt == 7),
                )
            nc.vector.tensor_scalar_add(QT[pair][0], q_ps, bq_sb[:, pair : pair + 1])

        # minimal prologue: just what unit (qc0, pair0, kt0) needs.  The
        # remaining projection groups are interleaved one-per-unit into the
        # early attention stream (their psum "av" rotation is free because AV
        # flushes are deferred until the projections complete).  Group order
        # follows DMA arrival so a stalled group never blocks a ready QK.
        q0_chunk(0, "av", 3)  # q0 inputs land ~2us before memc: keep PE fed
        k_chunk(0, 0, "av", 3)
        proj = [
            lambda: k_chunk(1, 0, "av", 3),
            lambda: q0_chunk(1, "av", 3),
        ]
        for c in range(1, n_mc):
            proj.append(lambda c=c: k_chunk(0, c, "av", 3))
            proj.append(lambda c=c: k_chunk(1, c, "av", 3))
        proj.extend(
            lambda kt=kt: v_chunk(kt, "av", 3) for kt in range(kt_tiles)
        )

        def q_stream_gen(qqc):
            # one q-chunk through the "work" psum slot; fine-grained 4-dt steps
            w, off = QW[qqc], QOFF[qqc]
            blk, boff = divmod(off, 512)
            for pair in range(2):
                cs = slice(128 * pair, 128 * pair + 128)
                q_ps = psum.tile([128, 512], F32, tag="work", bufs=1, name="q_ps")
                for dt in range(8):
                    nc.tensor.matmul(
                        q_ps[:, 0:w],
                        wq_sb[:, dt, cs],
                        xc[dt // 4][blk][:, dt % 4, boff : boff + w],
                        start=(dt == 0),
                        stop=(dt == 7),
                    )
                    if dt == 3:
                        yield
                nc.vector.tensor_scalar_add(
                    QT[pair][qqc], q_ps[:, 0:w], bq_sb[:, pair : pair + 1]
                )
                yield

        # --- attention: flat software-pipelined stream, pair-major ---------
        outT = [
            [
                qkv.tile([128, QW[q]], F16, tag=f"oT{p}_{q}", name=f"oT{p}_{q}")
                for q in range(NQ)
            ]
            for p in range(2)
        ]

        st_av = {}  # (qc, pair) -> [avA, avB] psum tiles [65, w]
        epi_count = [0]
        completed = []  # (qc, pair) whose final AV flush has been emitted
        nflush = {}  # (qc, pair) -> AV flushes emitted so far
        group_order = []  # (qc, pair) in first-push order
        group_seen = set()

        def flush(pend):
            # AV accumulation is order-independent over kt: start/stop flags
            # follow the per-pair flush COUNT, not the kt index.
            qc, pair, kt, p_t = pend
            w = QW[qc]
            n = nflush.get((qc, pair), 0)
            nflush[(qc, pair)] = n + 1
            av = st_av.setdefault((qc, pair), [None, None])
            for a in range(2):
                if av[a] is None:
                    av[a] = psum.tile(
                        [65, 512], F32, tag="av", bufs=3, name=f"av{pair}_{a}"
                    )[:, 0:w]
                nc.tensor.matmul(
                    av[a],
                    V[kt][:, 2 * pair + a, :],
                    p_t[:, a, :],
                    start=(n == 0),
                    stop=(n == kt_tiles - 1),
                )
            if n == kt_tiles - 1:
                completed.append((qc, pair))

        def drain_one(pending):
            # Flush only the oldest still-accumulating pair: at most two pairs
            # may hold av psum banks at once (3-slot rotation).
            g = next(
                (k for k in group_order if nflush.get(k, 0) < kt_tiles), None
            )
            idx = next(
                (i for i, p in enumerate(pending) if (p[0], p[1]) == g), None
            )
            if idx is None:
                return False
            flush(pending.pop(idx))
            return True

        def pair_epilogue(qc, pair):
            """1/s via Ln+Exp, E2-broadcast, fused normalize; out-proj on pair 1."""
            w, off = QW[qc], QOFF[qc]
            tail = qc == NQ - 1 and pair == 1  # nothing left to overlap with
            avA, avB = st_av[(qc, pair)]
            g = npool.tile([33, 512], F32, tag="g", bufs=4)
            if epi_count[0] < 4:
                nc.vector.memset(g, 1.0)  # keep Ln input positive in slack rows
            epi_count[0] += 1

            def outproj(sl, wtag, wbufs, ev_alt, cts=(0, 1)):
                o_sb = opool.tile([128, 1024], F16, tag="osb", name="o_sb")
                for nch in range(2):
                    o_ps = psum.tile(
                        [128, 512], F32,
                        tag=(wtag if nch % 2 else "work"),
                        bufs=(wbufs if nch % 2 else 1),
                        name="o_ps",
                    )
                    for j, ct in enumerate(cts):
                        nc.tensor.matmul(
                            o_ps,
                            outT[ct][qc][:, sl * 128 : (sl + 1) * 128],
                            wo_sb[:, ct, nch * 512 : (nch + 1) * 512],
                            start=(j == 0),
                            stop=(j == len(cts) - 1),
                        )
                    if ev_alt and nch % 2:
                        nc.scalar.copy(o_sb[:, nch * 512 : (nch + 1) * 512], o_ps)
                    else:
                        nc.vector.tensor_copy(
                            o_sb[:, nch * 512 : (nch + 1) * 512], o_ps
                        )
                return o_sb

            if tail:
                # final 256-q chunk: short serial chain, scalar engine takes
                # the gathers + odd evacs since it is idle here
                nc.scalar.copy(g[0:1, 0:w], avA[64:65, :])
                nc.scalar.copy(g[32:33, 0:w], avB[64:65, :])
                lns = npool.tile([33, w], F32, tag="lns", bufs=4)
                nc.scalar.activation(lns, g[:, 0:w], Ln)
                rcp16 = npool.tile([33, w], F16, tag="rcp16", bufs=4)
                nc.scalar.activation(rcp16, lns, Exp, scale=-1.0)
                bc_ps = psum.tile([128, w], F32, tag="work", bufs=1, name="bc_ps")
                nc.tensor.matmul(bc_ps, e2, rcp16, start=True, stop=True)
                bc_sb = npool.tile([128, w], F32, tag="bc_sb", bufs=4)
                nc.vector.tensor_copy(bc_sb, bc_ps)
                nc.vector.tensor_tensor(
                    outT[pair][qc][0:64, :], avA[0:64, :], bc_sb[0:64, :],
                    op=mybir.AluOpType.mult,
                )
                nc.vector.tensor_tensor(
                    outT[pair][qc][64:128, :], avB[0:64, :], bc_sb[64:128, :],
                    op=mybir.AluOpType.mult,
                )
                for sl in range(w // 128):
                    o_sb = outproj(sl, "lt", 2, ev_alt=True)
                    st = off // 128 + sl
                    eng = nc.scalar if sl % 2 else nc.sync
                    eng.dma_start(out=out_d[st * 128 : (st + 1) * 128, :], in_=o_sb)
                del st_av[(qc, pair)]
                return

            nc.vector.tensor_copy(g[0:1, 0:w], avA[64:65, :])
            nc.vector.tensor_copy(g[32:33, 0:w], avB[64:65, :])
            yield
            # 1/s on the DVE via fast-inverse seed + 2 Newton steps, keeping
            # the scalar engine free for the exp stream.  x0 = bitcast(MAGIC -
            # bitcast(s)); xk alternates sign: the final value is -1/s,
            # absorbed by the negated e2n selector.
            U32 = mybir.dt.uint32
            x0 = npool.tile([33, w], F32, tag="lns", bufs=4)
            nc.vector.tensor_tensor(
                x0.bitcast(U32), mgc[:, 0:w].bitcast(U32), g[:, 0:w].bitcast(U32),
                op=mybir.AluOpType.subtract,
            )
            t1 = npool.tile([33, w], F32, tag="nt", bufs=2)
            nc.vector.tensor_tensor(t1, g[:, 0:w], x0, op=mybir.AluOpType.mult)
            yield
            rcp16 = npool.tile([33, w], F16, tag="rcp16", bufs=4)
            nc.vector.scalar_tensor_tensor(
                rcp16, t1, 2.0, x0,
                op0=mybir.AluOpType.subtract, op1=mybir.AluOpType.mult,
            )
            yield
            bc_ps = psum.tile([128, w], F32, tag="work", bufs=1, name="bc_ps")
            nc.tensor.matmul(bc_ps, e2n, rcp16, start=True, stop=True)
            bc_sb = npool.tile([128, w], F32, tag="bc_sb", bufs=4)
            nc.vector.tensor_copy(bc_sb, bc_ps)
            yield
            nc.vector.tensor_tensor(
                outT[pair][qc][0:64, :], avA[0:64, :], bc_sb[0:64, :],
                op=mybir.AluOpType.mult,
            )
            nc.vector.tensor_tensor(
                outT[pair][qc][64:128, :], avB[0:64, :], bc_sb[64:128, :],
                op=mybir.AluOpType.mult,
            )
            del st_av[(qc, pair)]
            yield
            if pair == 0:
                return
            for sl in range(w // 128):
                o_sb = outproj(sl, "work", 1, ev_alt=False)
                yield
                st = off // 128 + sl
                nc.sync.dma_start(out=out_d[st * 128 : (st + 1) * 128, :], in_=o_sb)

        # qc0's units run during the projection interleave, so they follow
        # memT chunk arrival (kt blocks of 3, both pairs per block); later
        # chunks are pair-major as before.
        units = [
            (0, pair, 3 * blk + k)
            for blk in range(n_mc)
            for pair in range(2)
            for k in range(3)
        ]
        units += [
            (qc, pair, kt)
            for qc in range(1, NQ)
            for pair in range(2)
            for kt in range(kt_tiles)
        ]

        def _step2(gens):
            # two steps per unit: halves epilogue wall-latency so av psum
            # banks recycle sooner at pair transitions
            out = []
            for g_ in gens:
                if next(g_, StopIteration) is StopIteration:
                    continue
                if next(g_, StopIteration) is StopIteration:
                    continue
                out.append(g_)
            return out

        prio = []
        # q-chunk streams gated into the PE-slack zone (chunk qi's QT is
        # needed at unit qi*18); chunk 1 waits for its xc DMA (~unit 3).
        qgens = [
            (q_stream_gen(1), 8),
            (q_stream_gen(2), 22),
            (q_stream_gen(3), 40),
        ]
        pending = []
        for u, (qc, pair, kt) in enumerate(units):
            # QK first: the tensor queue head must be the op that feeds the
            # exp stream, never something that waits on it.
            w = QW[qc]
            ks = slice(kt * 128, (kt + 1) * 128)
            # psum accumulation groups must stay bank-aligned: keep the two
            # QK halves in separate 2KB banks even for 256-wide chunks
            lt_full = psum.tile([128, 2, 512], F32, tag="lt", bufs=2, name="lt")
            lt = lt_full[:, :, 0:w]
            nc.tensor.matmul(
                lt[:, 0, :], KT[pair][0:64, ks], QT[pair][qc][0:64, :],
                start=True, stop=True,
            )
            nc.tensor.matmul(
                lt[:, 1, :], KT[pair][64:128, ks], QT[pair][qc][64:128, :],
                start=True, stop=True,
            )
            p_full = ppool.tile([128, 2, 512], F16, tag="p")
            p_t = p_full[:, :, 0:w]
            nc.scalar.activation(
                p_t, lt, Exp, bias=mb_sb[:, kt : kt + 1], scale=0.125
            )
            if (qc, pair) not in group_seen:
                group_seen.add((qc, pair))
                group_order.append((qc, pair))
            pending.append((qc, pair, kt, p_t))
            if proj:
                # one deferred projection group per unit; AV flushes wait
                # (ppool is deep enough to hold the p_t backlog)
                proj.pop(0)()
            else:
                # drain the flush backlog at up to 2/unit, then steady-state
                # with a 4-unit lag (the lag keeps the in-order tensor queue
                # from stalling on an exp still in flight)
                drains = 0
                while len(pending) > 4 and drains < 2:
                    if not drain_one(pending):
                        break
                    drains += 1
            for key in completed:
                prio.append(pair_epilogue(*key))
            completed.clear()
            while qgens and qgens[0][1] <= u:
                if next(qgens[0][0], StopIteration) is StopIteration:
                    qgens.pop(0)
                break
            prio = _step2(prio)

        while pending:
            assert drain_one(pending)
            for key in completed:
                prio.append(pair_epilogue(*key))
            completed.clear()
            prio = _step2(prio)
        for gen, _ in qgens:
            for _ in gen:
                pass
        for gen in prio:
            for _ in gen:
                pass

    return nc


_PROG_CACHE: dict[int, bass.Bass] = {}


def _get_prog(k_pad: int) -> bass.Bass:
    if k_pad not in _PROG_CACHE:
        _PROG_CACHE[k_pad] = _build(k_pad)
    return _PROG_CACHE[k_pad]


def _shuf_w(w):
    # [1024, CD] -> [128, 8, CD]: sbuf partition p holds rows 128*t + p
    return np.ascontiguousarray(w.astype(np.float16).reshape(8, 128, -1).transpose(1, 0, 2))


def _shuf_chunks(tT, csz):
    # [D, N] -> [2, N/csz, 128, 4, csz] contiguous DMA slabs per (half, chunk)
    d, n = tT.shape
    a = tT.reshape(2, 4, 128, n // csz, csz)  # [half, t, p, chunk, csz]
    return np.ascontiguousarray(a.transpose(0, 3, 2, 1, 4))


def _prep_inputs(x, memory, mask, wq, bq, wk, wv, k_pad):
    """Build the 8 per-core input maps."""
    kt_tiles = k_pad // 128
    in_maps = []
    per_batch = []
    for b in range(B):
        idx = np.flatnonzero(~mask[b])
        n = len(idx)
        assert n <= k_pad
        mem_g = np.zeros((k_pad, D), np.float16)
        mem_g[:n] = memory[b][idx].astype(np.float16)
        memT_b = _shuf_chunks(np.ascontiguousarray(mem_g.T), MEMC)
        xT_b = _shuf_chunks(np.ascontiguousarray(x[b].astype(np.float16).T), 512)
        mbias = np.zeros(k_pad, np.float32)
        mbias[n:] = NEG
        mb_b = np.ascontiguousarray(mbias.reshape(kt_tiles, 128).T)
        per_batch.append((xT_b, memT_b, mb_b, idx))
    for c in range(N_CORES):
        b, g = divmod(c, G)
        xT_b, memT_b, mb_b, _ = per_batch[b]
        cs = slice(g * CD, (g + 1) * CD)
        in_maps.append(
            {
                "xT": xT_b,
                "memT": memT_b,
                "wq": _shuf_w(wq[:, cs]),
                "wk": _shuf_w(wk[:, cs]),
                "wv": _shuf_w(wv[:, cs]),
                "bq": np.ascontiguousarray(bq[cs].reshape(2, 128).T.astype(np.float32)),
                "wo": None,  # filled by caller (needs wo)
                "maskb": mb_b,
            }
        )
    return in_maps, per_batch


def kernel(x, memory, mask, wq, bq, wk, bk, wv, bv, wo, bo, _trace=False):
    x = np.asarray(x, np.float32)
    memory = np.asarray(memory, np.float32)
    mask = np.asarray(mask).astype(bool)
    wq = np.asarray(wq, np.float32)
    bq = np.asarray(bq, np.float32)
    wk = np.asarray(wk, np.float32)
    wv = np.asarray(wv, np.float32)
    bv = np.asarray(bv, np.float32)
    wo = np.asarray(wo, np.float32)
    bo = np.asarray(bo, np.float32)

    nmax = max(int((~mask[b]).sum()) for b in range(B))
    k_pad = next(k for k in K_PAD_LADDER if k >= nmax)
    prog = _get_prog(k_pad)

    in_maps, _ = _prep_inputs(x, memory, mask, wq, bq, wk, wv, k_pad)
    for c in range(N_CORES):
        g = c % G
        in_maps[c]["wo"] = np.ascontiguousarray(
            wo[g * CD : (g + 1) * CD, :].astype(np.float16).reshape(2, 128, D).transpose(1, 0, 2)
        )

    res = run_bass_kernel_spmd(prog, in_maps, list(range(N_CORES)), trace=_trace)
    outs = [res.results[c]["out"] for c in range(N_CORES)]
    final = np.empty((B, S, D), np.float32)
    tail = bo + bv @ wo
    for b in range(B):
        final[b] = outs[G * b].astype(np.float32)
        for g in range(1, G):
            final[b] += outs[G * b + g].astype(np.float32)
        final[b] += tail[None, :]
    if _trace:
        kernel.last_exec_time_ns = res.exec_time_ns
    return final

